# revision 3
# baseline (speedup 1.0000x reference)
"""CapsuleLayer kernel for 8x Trainium2 NeuronCores.

Reference computes h = x @ W[0]  ([32,512]@[512,16384] -> [32,256,64] f32)
followed by 3 "routing" rounds:
    c = softmax(h, axis=1); h = einsum('bid,bjd->bjd', c, h)
The einsum contracts i only over c, so it equals h * sum_i c[b,i,d] = h * 1
(softmax sums to one over the contracted axis) -- the routing loop is the
identity up to f32 rounding (~1e-7 relative). The kernel therefore computes
just the matmul, sharded over the 16384-wide output dim across 8 cores so
each core streams a distinct 4 MiB slice of W (memory-bound roofline).

Numerics: inputs are split on the host into fp16 hi/lo pairs
(x = xh + xl, W = wh + wl exactly to ~2^-22 relative) and the product is
computed as xh@wh + xh@wl + xl@wh on the PE at full fp16 rate with fp32
PSUM accumulation -- fp32-class accuracy at 4x the fp32 matmul throughput,
with the same 4 bytes/element of HBM traffic.

Raw Bass (no TileContext) with a hand-rolled feed-forward pipeline: every
buffer is written exactly once, so the only semaphores are the natural
producer->consumer edges and there is no drain/barrier tail.
"""

import numpy as np

B = 32          # batch
K = 512         # in_dim (contraction)
N_FULL = 16384  # num_capsules * out_dim
NUM_CAPS = 256
OUT_DIM = 64
NUM_CORES = 8
N_SHARD = N_FULL // NUM_CORES  # 2048 columns per core

KI = 128            # contraction partition tile
KO = K // KI        # 4 contraction subtiles
NT = 512            # output-column chunk (= max fp32-PSUM bank free dim)
NCH = N_SHARD // NT  # 4 chunks per core
N_WARM = 8          # PE warmup matmuls (HAM clock-gate ramp)

_NC = None
LAST_RESULTS = None  # BassKernelResults of the most recent run (for profiling)


def _build_nc():
    import concourse.bass as bass
    import concourse.mybir as mybir

    f16 = mybir.dt.float16
    f32 = mybir.dt.float32
    nc = bass.Bass("TRN2", target_bir_lowering=False)

    # Host-prepacked fp16 hi/lo pairs, contiguous per partition:
    #  xp[ki, hl, ko, b]     = split(x)[hl][b, ko*KI + ki]
    #  wp[j, ki, hl, ko, t]  = split(W)[hl][ko*KI + ki, n0 + j*NT + t]
    xp = nc.dram_tensor("xp", [KI, 2 * KO * B], f16, kind="ExternalInput")
    wp = nc.dram_tensor("wp", [NCH, KI, 2 * KO * NT], f16, kind="ExternalInput")
    out = nc.dram_tensor("out", [B, N_SHARD], f32, kind="ExternalOutput")

    x_tile = nc.alloc_sbuf_tensor("x_tile", [KI, 2 * KO * B], f16)
    w_tiles = [
        nc.alloc_sbuf_tensor(f"w_tile{j}", [KI, 2 * KO * NT], f16)
        for j in range(NCH)
    ]
    o_tiles = [nc.alloc_sbuf_tensor(f"o_tile{j}", [B, NT], f32) for j in range(NCH)]
    warm_tile = nc.alloc_sbuf_tensor("warm_tile", [KI, NT], f16)

    ps_tiles = [nc.alloc_psum_tensor(f"ps{j}", [B, NT], f32) for j in range(NCH)]
    ps_warm = nc.alloc_psum_tensor("ps_warm", [B, NT], f32)

    x_ap = x_tile.ap().rearrange("ki (hl ko b) -> ki hl ko b", hl=2, ko=KO)
    w_aps = [
        w.ap().rearrange("ki (hl ko t) -> ki hl ko t", hl=2, ko=KO) for w in w_tiles
    ]
    # (x_half, w_half) product terms: hh + hl + lh (ll ~ 2^-44, dropped)
    TERMS = [(0, 0), (0, 1), (1, 0)]

    x_sem = nc.alloc_semaphore("x_sem")
    w_sem = nc.alloc_semaphore("w_sem")
    warm_sem = nc.alloc_semaphore("warm_sem")
    mm_sem = nc.alloc_semaphore("mm_sem")
    cp_sem = nc.alloc_semaphore("cp_sem")
    out_sem = nc.alloc_semaphore("out_sem")

    with nc.Block() as block:

        @block.gpsimd
        def _(gpsimd):
            gpsimd.memset(warm_tile[:], 0).then_inc(warm_sem, 1)
            gpsimd.dma_start(x_tile[:], xp[:]).then_inc(x_sem, 16)

        @block.sync
        def _(sync):
            for j in range(NCH):
                sync.dma_start(w_tiles[j][:], wp[j]).then_inc(w_sem, 16)

        @block.tensor
        def _(tensor):
            tensor.wait_ge(warm_sem, 1)
            for _ in range(N_WARM):
                tensor.matmul(
                    ps_warm[:], warm_tile[:, :B], warm_tile[:], start=True, stop=True
                )
            tensor.wait_ge(x_sem, 16)
            for j in range(NCH):
                tensor.wait_ge(w_sem, 16 * (j + 1))
                for i, (xh, wh) in enumerate(TERMS):
                    for ko in range(KO):
                        ins = tensor.matmul(
                            ps_tiles[j][:],
                            x_ap[:, xh, ko, :],
                            w_aps[j][:, wh, ko, :],
                            start=(i == 0 and ko == 0),
                            stop=(i == len(TERMS) - 1 and ko == KO - 1),
                        )
                ins.then_inc(mm_sem, 1)

        @block.vector
        def _(vector):
            for j in range(NCH):
                vector.wait_ge(mm_sem, j + 1)
                vector.tensor_copy(o_tiles[j][:], ps_tiles[j][:]).then_inc(cp_sem, 1)

        @block.scalar
        def _(scalar):
            for j in range(NCH):
                scalar.wait_ge(cp_sem, j + 1)
                scalar.dma_start(
                    out[:, j * NT : (j + 1) * NT], o_tiles[j][:]
                ).then_inc(out_sem, 16)
            scalar.wait_ge(out_sem, 16 * NCH)

    return nc


def _get_nc():
    global _NC
    if _NC is None:
        _NC = _build_nc()
    return _NC


def _split_f16(a):
    hi = a.astype(np.float16)
    lo = (a - hi.astype(np.float32)).astype(np.float16)
    return hi, lo


def kernel(x, W):
    global LAST_RESULTS
    from concourse.bass_utils import run_bass_kernel_spmd

    x = np.ascontiguousarray(np.asarray(x, dtype=np.float32))
    W2 = np.ascontiguousarray(np.asarray(W, dtype=np.float32)).reshape(K, N_FULL)

    xh, xl = _split_f16(x)
    wh, wl = _split_f16(W2)

    # xp[ki, hl, ko, b] = x_hl[b, ko*KI + ki]  -> [KI, 2*KO*B]
    xs = np.stack([xh, xl])  # [2, B, K]
    xp = np.ascontiguousarray(
        xs.transpose(2, 0, 1).reshape(KO, KI, 2, B).transpose(1, 2, 0, 3).reshape(
            KI, 2 * KO * B
        )
    )
    # wfull[jf, ki, hl, ko, t] = w_hl[ko*KI + ki, jf*NT + t]
    jf_total = N_FULL // NT
    ws = np.stack([wh, wl])  # [2, K, N]
    wfull = np.ascontiguousarray(
        ws.reshape(2, KO, KI, jf_total, NT).transpose(3, 2, 0, 1, 4).reshape(
            jf_total, KI, 2 * KO * NT
        )
    )

    nc = _get_nc()
    in_maps = []
    for c in range(NUM_CORES):
        in_maps.append({"xp": xp, "wp": wfull[c * NCH : (c + 1) * NCH]})

    res = run_bass_kernel_spmd(nc, in_maps, core_ids=list(range(NUM_CORES)))
    LAST_RESULTS = res
    full = np.concatenate([r["out"] for r in res.results], axis=1)
    return full.reshape(B, NUM_CAPS, OUT_DIM)
